# revision 1
# baseline (speedup 1.0000x reference)
"""Segment-mean pooling (segment_sum / counts) + Linear, on 8 TRN2 NeuronCores.

Strategy: segment-ownership sharding.  The host sorts rows by dst_idx and
routes each row to the core that owns its segment range (core i owns
segments [512*i, 512*(i+1))), so no collectives are needed; the host
concatenates the 8 output shards.

Per core, the segment sums are computed in [segment, hidden] layout
(segments on PSUM partitions) in two passes:

  Pass 1 (banded): the host packs the first C=16 rows of every segment
  into a dense band of 16-row slots (~98% full).  A 128-row chunk then
  covers exactly 8 consecutive segments, and its segment-sum is ONE
  TensorE matmul: stationary = a constant block-ones [128, 32] matrix,
  moving = the x rows [128, 256].  No per-row index handling at all.

  Pass 2 (one-hot tail): rows beyond slot 16 (~4% of rows) go through
  windowed one-hot matmuls: VectorE builds is_equal one-hots against an
  iota row (precomputed during pass 1), and each chunk's matmuls write
  narrow 32-aligned windows of the accumulators.  The window schedule is
  shared across cores (min/max over cores) so the SPMD graph is
  identical on every core.

Both band and overflow arrays are shipped pre-swizzled as [128, k, 256]
so every DMA is a fully linear copy.  PSUM accumulators are zero-opened
by rank-1 matmuls, so all data matmuls are pure accumulates in any
order.  Every PSUM tensor is padded to a full private 2 KiB bank, and
VectorE only reads a bank once all TensorE writes to it are complete
(PE-write + DVE-read on one bank is a fatal HW error).

Epilogue: scale rows by 1/(count+eps) (host bincount shipped as a
reciprocal table), PE-transpose pooled to [hidden, segment], apply the
Linear as out[s, j] = pooled_T[:, s].T @ W.T[h, j] with fused bias-add
(per-tile pipelined), and DMA the [512, 256] f32 shard.
"""

import os

import numpy as np

import concourse.bass as bass
import concourse.mybir as mybir
from concourse.bass_utils import run_bass_kernel_spmd

N_CORES = 8
S_TOTAL = 4096
S_PER = S_TOTAL // N_CORES  # 512 segments per core
H = 256
EPS = np.float32(1e-8)
PAD_IDX = 9999.0  # sentinel relative idx; never matches iota [0, wmax2)
C = 16  # band-A capacity (rows per segment); must divide 128
C2 = 8  # band-B capacity (rows 16..24 of a segment); must divide 128

GSZ = 8  # chunks per band DMA (1024 rows, 512 KB)
KB = S_PER * C // 128  # 64 band-A chunks
KB2 = S_PER * C2 // 128  # 32 band-B chunks
N_BAND_GROUPS = KB // GSZ  # 8
N_B2_GROUPS = KB2 // GSZ  # 4

_graph_cache: dict = {}

if os.environ.get("K_LDW"):
    try:
        import libneuronxla.libncc as _ncc

        _ncc.NEURON_CC_FLAGS = [
            f.replace("--enable-ldw-opt=false", "--enable-ldw-opt=true")
            for f in _ncc.NEURON_CC_FLAGS
        ]
        os.environ["AXON_NCC_FLAGS"] = os.environ.get("AXON_NCC_FLAGS", "").replace(
            "--enable-ldw-opt=false", "--enable-ldw-opt=true"
        )
    except Exception:
        pass


def _build(ov_chunks: int, ov_parts: tuple, wmax2: int) -> "bass.Bass":
    """ov_parts[oc] = tuple of 32-aligned window-part start segments."""
    f16 = mybir.dt.float16
    f32 = mybir.dt.float32
    ovk = max(ov_chunks, 1)

    nc = bass.Bass()

    xb_d = nc.declare_dram_parameter("xb", [128, KB, H], f16, isOutput=False)
    xb2_d = nc.declare_dram_parameter("xb2", [128, KB2, H], f16, isOutput=False)
    xov_d = nc.declare_dram_parameter("xov", [128, ovk, H], f16, isOutput=False)
    ovidx_d = nc.declare_dram_parameter("ovidx", [128, ovk], f32, isOutput=False)
    iota_d = nc.declare_dram_parameter("iota", [128, wmax2 + 256], f16, isOutput=False)
    ones_d = nc.declare_dram_parameter("ones32", [128, 6, 32], f16, isOutput=False)
    ident_d = nc.declare_dram_parameter("ident", [128, 128], f16, isOutput=False)
    wt_d = nc.declare_dram_parameter("wt", [H, H], f16, isOutput=False)
    invc_d = nc.declare_dram_parameter("invc", [128, 4], f32, isOutput=False)
    bb_d = nc.declare_dram_parameter("bb", [128, H], f32, isOutput=False)
    out_d = nc.declare_dram_parameter("out", [S_PER, H], f32, isOutput=True)

    from contextlib import ExitStack

    with ExitStack() as ctx:
        xbb = ctx.enter_context(nc.sbuf_tensor("xbb", [128, KB, H], f16))
        xbb2 = ctx.enter_context(nc.sbuf_tensor("xbb2", [128, KB2, H], f16))
        xov_sb = ctx.enter_context(nc.sbuf_tensor("xov_sb", [128, ovk, H], f16))
        oh2 = ctx.enter_context(nc.sbuf_tensor("oh2", [128, ovk, wmax2], f16))
        ovidx_sb = ctx.enter_context(nc.sbuf_tensor("ovidx_sb", [128, ovk], f32))
        iota_sb = ctx.enter_context(nc.sbuf_tensor("iota_sb", [128, wmax2 + 256], f16))
        ones_sb = ctx.enter_context(nc.sbuf_tensor("ones_sb", [128, 6, 32], f16))
        ident_sb = ctx.enter_context(nc.sbuf_tensor("ident_sb", [128, 128], f16))
        wt_sb = ctx.enter_context(nc.sbuf_tensor("wt_sb", [128, 2, H], f16))
        invc_sb = ctx.enter_context(nc.sbuf_tensor("invc_sb", [128, 4], f32))
        bb_sb = ctx.enter_context(nc.sbuf_tensor("bb_sb", [128, H], f32))
        pool_sb = ctx.enter_context(nc.sbuf_tensor("pool_sb", [128, 4, H], f16))
        sums2_sb = ctx.enter_context(nc.sbuf_tensor("sums2_sb", [128, 2, S_PER], f16))
        out_sb = ctx.enter_context(nc.sbuf_tensor("out_sb", [128, 4, H], f32))
        # every PSUM tensor padded to one full private 2 KiB bank
        ps_s = [
            ctx.enter_context(nc.psum_tensor(f"ps_s{t}", [128, 512], f32))
            for t in range(4)
        ]
        ps_t = [
            ctx.enter_context(nc.psum_tensor(f"ps_t{hb}", [128, 1024], f16))
            for hb in range(2)
        ]
        ps_x = ctx.enter_context(nc.psum_tensor("ps_x", [128, 512], f32))
        dma_sem = ctx.enter_context(nc.semaphore("dma_sem"))
        csem = {
            name: ctx.enter_context(nc.semaphore(f"csem_{name}"))
            for name in ("iota", "ovidx", "ones", "ident", "wt", "invc", "bb")
        }
        bsem = [
            ctx.enter_context(nc.semaphore(f"bsem{g}"))
            for g in range(N_BAND_GROUPS)
        ]
        b2sem = [
            ctx.enter_context(nc.semaphore(f"b2sem{g}"))
            for g in range(N_B2_GROUPS)
        ]
        xsem = ctx.enter_context(nc.semaphore("xsem"))
        b2last = ctx.enter_context(nc.semaphore("b2last"))
        cmp_sem = ctx.enter_context(nc.semaphore("cmp_sem"))
        mm_sem = ctx.enter_context(nc.semaphore("mm_sem"))
        cp_sem = ctx.enter_context(nc.semaphore("cp_sem"))
        tr_sem = ctx.enter_context(nc.semaphore("tr_sem"))
        cp2_sem = ctx.enter_context(nc.semaphore("cp2_sem"))
        mme_sem = ctx.enter_context(nc.semaphore("mme_sem"))
        oe_sem = ctx.enter_context(nc.semaphore("oe_sem"))
        block = ctx.enter_context(nc.Block())

        zlhs = iota_sb[0:1, 0:128]  # junk values; multiplied by zero rhs
        zrhs = iota_sb[0:1, wmax2 : wmax2 + 256]  # zeros

        @block.sync
        def _(sync):
            # late-needed consts on the sync ring
            sync.dma_start(out=ident_sb[:, :], in_=ident_d[:, :]).then_inc(
                csem["ident"], 16
            )
            sync.dma_start(
                out=wt_sb[:, :, :],
                in_=wt_d[:, :].rearrange("(t p) j -> p t j", p=128),
            ).then_inc(csem["wt"], 16)
            sync.dma_start(out=invc_sb[:, :], in_=invc_d[:, :]).then_inc(
                csem["invc"], 16
            )
            sync.dma_start(out=bb_sb[:, :], in_=bb_d[:, :]).then_inc(csem["bb"], 16)
            for st in range(4):
                sync.wait_ge(oe_sem, st + 1)
                sync.dma_start(
                    out=out_d[st * 128 : (st + 1) * 128, :], in_=out_sb[:, st, :]
                ).then_inc(dma_sem, 16)
            for name in ("ident", "wt", "invc", "bb"):
                sync.wait_ge(csem[name], 16)
            sync.wait_ge(dma_sem, 16 * 4)

        @block.scalar
        def _(scalar):
            # ALL input DMAs on one ring, in consumption order, one
            # semaphore per DMA: cumulative thresholds on a shared sem
            # can't tell WHICH transfer completed.
            scalar.dma_start(out=ones_sb[:, :, :], in_=ones_d[:, :, :]).then_inc(
                csem["ones"], 16
            )
            scalar.dma_start(out=iota_sb[:, :], in_=iota_d[:, :]).then_inc(
                csem["iota"], 16
            )
            scalar.dma_start(out=ovidx_sb[:, :], in_=ovidx_d[:, :]).then_inc(
                csem["ovidx"], 16
            )
            scalar.dma_start(out=xov_sb[:, :, :], in_=xov_d[:, :, :]).then_inc(
                xsem, 16
            )
            for g in range(N_BAND_GROUPS):
                scalar.dma_start(
                    out=xbb[:, GSZ * g : GSZ * (g + 1), :],
                    in_=xb_d[:, GSZ * g : GSZ * (g + 1), :],
                ).then_inc(bsem[g], 16)
            for g in range(N_B2_GROUPS - 1):
                scalar.dma_start(
                    out=xbb2[:, GSZ * g : GSZ * (g + 1), :],
                    in_=xb2_d[:, GSZ * g : GSZ * (g + 1), :],
                ).then_inc(b2sem[g], 16)
            gl = N_B2_GROUPS - 1
            scalar.dma_start(
                out=xbb2[:, GSZ * gl : GSZ * gl + 4, :],
                in_=xb2_d[:, GSZ * gl : GSZ * gl + 4, :],
            ).then_inc(b2sem[gl], 16)
            scalar.dma_start(
                out=xbb2[:, GSZ * gl + 4 : GSZ * (gl + 1), :],
                in_=xb2_d[:, GSZ * gl + 4 : GSZ * (gl + 1), :],
            ).then_inc(b2last, 16)
            for g in range(N_BAND_GROUPS):
                scalar.wait_ge(bsem[g], 16)
            for g in range(N_B2_GROUPS - 1):
                scalar.wait_ge(b2sem[g], 16)
            scalar.wait_ge(b2sem[N_B2_GROUPS - 1], 16)
            scalar.wait_ge(b2last, 16)
            scalar.wait_ge(xsem, 16)
            for name in ("ones", "iota", "ovidx"):
                scalar.wait_ge(csem[name], 16)

        @block.vector
        def _(vector):
            # pass-2 one-hots, precomputed while PE runs the band pass
            if ov_chunks:
                vector.wait_ge(csem["iota"], 16)
                vector.wait_ge(csem["ovidx"], 16)
                for oc in range(ov_chunks):
                    woc = 32 * len(ov_parts[oc])
                    vector.tensor_scalar(
                        out=oh2[:, oc, 0:woc],
                        in0=iota_sb[:, 0:woc],
                        scalar1=ovidx_sb[:, oc : oc + 1],
                        scalar2=None,
                        op0=mybir.AluOpType.is_equal,
                    ).then_inc(cmp_sem, 1)
            # epilogue
            vector.wait_ge(mm_sem, 1)  # all accumulation done
            for st in range(4):
                vector.tensor_copy(
                    out=pool_sb[:, st, :], in_=ps_s[st][:, 0:H]
                ).then_inc(cp_sem, 1)
            # ps_t banks are PE-owned until ALL transposes finish
            vector.wait_ge(tr_sem, 4)
            for st in range(4):
                vector.tensor_copy(
                    out=sums2_sb[:, 0, 128 * st : 128 * (st + 1)],
                    in_=ps_t[0][:, 128 * st : 128 * (st + 1)],
                )
                vector.tensor_copy(
                    out=sums2_sb[:, 1, 128 * st : 128 * (st + 1)],
                    in_=ps_t[1][:, 128 * st : 128 * (st + 1)],
                ).then_inc(cp2_sem, 1)
            vector.wait_ge(csem["invc"], 16)
            vector.wait_ge(csem["bb"], 16)
            for st in range(4):
                vector.wait_ge(mme_sem, st + 1)
                vector.scalar_tensor_tensor(
                    out=out_sb[:, st, :],
                    in0=ps_s[st][:, 0:H],
                    scalar=invc_sb[:, st : st + 1],
                    in1=bb_sb[:, :],
                    op0=mybir.AluOpType.mult,
                    op1=mybir.AluOpType.add,
                ).then_inc(oe_sem, 1)

        @block.tensor
        def _(tensor):
            tensor.wait_ge(csem["iota"], 16)
            tensor.wait_ge(csem["ones"], 16)
            # warm the PE clock (HAM) while the first data DMAs are in
            # flight: ~3.4us of sustained matmul activity moves the PE
            # from 1.2 GHz to 2.4 GHz for the whole band pass
            for _ in range(14):
                tensor.matmul(
                    ps_x[:, 0:256], ident_sb[:, :], iota_sb[:, 0:256],
                    start=True, stop=True, skip_group_check=True,
                )
            # zero-open all four accumulators
            for t in range(4):
                tensor.matmul(
                    ps_s[t][:, 0:H], zlhs, zrhs, start=True, stop=False,
                    skip_group_check=True,
                )
            # overflow one-hot pass first: runs while the bands stream in
            if ov_chunks:
                tensor.wait_ge(xsem, 16)
                for oc in range(ov_chunks):
                    tensor.wait_ge(cmp_sem, oc + 1)
                    for pi, seg0 in enumerate(ov_parts[oc]):
                        t, poff = seg0 // 128, seg0 % 128
                        tensor.matmul(
                            ps_s[t][poff : poff + 32, 0:H],
                            oh2[:, oc, 32 * pi : 32 * (pi + 1)],
                            xov_sb[:, oc, :],
                            start=False,
                            stop=False,
                            skip_group_check=True,
                            tile_position=(0, poff),
                        )
            # band A: chunk c covers segs [8c, 8c+8)
            for c in range(KB):
                if c % GSZ == 0:
                    tensor.wait_ge(bsem[c // GSZ], 16)
                    # full-width pulse so the HAM keeps the PE at 2.4 GHz
                    # (M=32 band matmuls alone do not register as busy)
                    tensor.matmul(
                        ps_x[:, 0:512], ident_sb[:, :],
                        xbb[:, GSZ * (c // GSZ) : GSZ * (c // GSZ) + 2, :],
                        start=True, stop=True, skip_group_check=True,
                    )
                v, j = divmod(c, 16)
                t, poff = j // 4, 32 * (j % 4)
                tensor.matmul(
                    ps_s[t][poff : poff + 32, 0:H],
                    ones_sb[:, v, :],
                    xbb[:, c, :],
                    start=False,
                    stop=False,
                    skip_group_check=True,
                    tile_position=(0, poff),
                )
            # band B: chunk c covers segs [16c, 16c+16)
            for c in range(KB2):
                g2 = c // GSZ
                if c % GSZ == 0:
                    tensor.wait_ge(b2sem[g2], 16)
                    tensor.matmul(
                        ps_x[:, 0:256], ident_sb[:, :], iota_sb[:, 0:256],
                        start=True, stop=True, skip_group_check=True,
                    )
                if g2 == N_B2_GROUPS - 1 and c % GSZ == 4:
                    tensor.wait_ge(b2last, 16)
                u, j = divmod(c, 16)
                t, poff = j // 4, 32 * (j % 4)
                tensor.matmul(
                    ps_s[t][poff : poff + 32, 0:H],
                    ones_sb[:, 4 + u, :],
                    xbb2[:, c, :],
                    start=False,
                    stop=False,
                    skip_group_check=True,
                    tile_position=(0, poff),
                )
            # close the accumulators
            for t in range(4):
                tensor.matmul(
                    ps_s[t][:, 0:H], zlhs, zrhs, start=False, stop=True,
                    skip_group_check=True,
                )
            # fence: matmul ends are FIFO; a matmul's then_inc can fire
            # before its PSUM writes drain, so hand banks to DVE only
            # after a trailing fence matmul completes
            tensor.matmul(
                ps_x[:, 0:H], zlhs, zrhs, start=True, stop=True,
                skip_group_check=True,
            ).then_inc(mm_sem, 1)
            # transposes: pooled [s, h] -> pooled_T [h, s], per tile
            tensor.wait_ge(csem["ident"], 16)
            for st in range(4):
                tensor.wait_ge(cp_sem, st + 1)
                for hb in range(2):
                    ins = tensor.transpose(
                        ps_t[hb][:, 128 * st : 128 * (st + 1)],
                        pool_sb[:, st, 128 * hb : 128 * (hb + 1)],
                        ident_sb[:, :],
                    )
                if st < 3:
                    ins.then_inc(tr_sem, 1)
                else:
                    tensor.matmul(
                        ps_x[:, 0:H], zlhs, zrhs, start=True, stop=True,
                        skip_group_check=True,
                    ).then_inc(tr_sem, 1)
            # Linear: out[s, j] = sum_h pooled_T[h, s] * wt[h, j]
            tensor.wait_ge(csem["wt"], 16)
            for st in range(4):
                tensor.wait_ge(cp2_sem, st + 1)
                tensor.matmul(
                    ps_s[st][:, 0:H],
                    sums2_sb[:, 0, st * 128 : (st + 1) * 128],
                    wt_sb[:, 0, :],
                    start=True,
                    stop=False,
                )
                tensor.matmul(
                    ps_s[st][:, 0:H],
                    sums2_sb[:, 1, st * 128 : (st + 1) * 128],
                    wt_sb[:, 1, :],
                    start=False,
                    stop=True,
                )
                tensor.matmul(
                    ps_x[:, 0:H], zlhs, zrhs, start=True, stop=True,
                    skip_group_check=True,
                ).then_inc(mme_sem, 1)

    return nc


def kernel(x, dst_idx, dst_size, W, b):
    x = np.asarray(x)
    idx = np.asarray(dst_idx).astype(np.int64)
    W = np.asarray(W, dtype=np.float32)
    b = np.asarray(b, dtype=np.float32)
    S = int(dst_size)
    assert S == S_TOTAL and x.shape[1] == H

    counts = np.bincount(idx, minlength=S).astype(np.float32)
    inv = np.float32(1.0) / (counts + EPS)  # [4096] f32

    order = np.argsort(idx, kind="stable")
    sidx = idx[order]
    bounds = np.searchsorted(sidx, np.arange(0, S + 1, S_PER))

    x16 = x.astype(np.float16)

    # split each core's rows into band A (rank < C), band B
    # (C <= rank < C+C2), and overflow (rank >= C+C2)
    bands, bands2, ovs, ovsegs = [], [], [], []
    for i in range(N_CORES):
        lo_i, hi_i = bounds[i], bounds[i + 1]
        n_i = hi_i - lo_i
        li = (sidx[lo_i:hi_i] - S_PER * i).astype(np.int64)
        rows = order[lo_i:hi_i]
        starts = np.searchsorted(li, np.arange(S_PER + 1))
        rank = np.arange(n_i) - starts[li]
        bm = rank < C
        sa = li[bm]
        slot = (16 * ((sa % 32) // 8) + sa // 32) * 128 + (sa % 8) * C + rank[bm]
        xband = np.zeros((128, KB, H), dtype=np.float16)
        xband[slot % 128, slot // 128] = x16[rows[bm]]
        bands.append(xband)
        bm2 = (rank >= C) & (rank < C + C2)
        sb = li[bm2]
        slot2 = (16 * ((sb % 32) // 16) + sb // 32) * 128 + (sb % 16) * C2 + (
            rank[bm2] - C
        )
        xband2 = np.zeros((128, KB2, H), dtype=np.float16)
        xband2[slot2 % 128, slot2 // 128] = x16[rows[bm2]]
        bands2.append(xband2)
        om = rank >= C + C2
        ovs.append(x16[rows[om]])
        ovsegs.append(li[om])

    ov_chunks = max(-(-len(s) // 128) for s in ovsegs)
    ovk = max(ov_chunks, 1)

    # shared overflow window schedule (32-aligned part starts)
    wins, parts = [], []
    for oc in range(ov_chunks):
        lo_w, hi_w = S_PER - 1, 0
        for s in ovsegs:
            seg = s[128 * oc : 128 * (oc + 1)]
            if len(seg):
                lo_w = min(lo_w, int(seg[0]))
                hi_w = max(hi_w, int(seg[-1]))
        hi_w = max(hi_w, lo_w)
        w = (lo_w // 32) * 32
        wins.append(w)
        parts.append(tuple(range(w, (hi_w // 32) * 32 + 32, 32)))
    wmax2 = max((len(p) for p in parts), default=1) * 32
    parts_t = tuple(parts)

    key = (ov_chunks, parts_t, wmax2)
    nc = _graph_cache.get(key)
    if nc is None:
        nc = _build(ov_chunks, parts_t, wmax2)
        _graph_cache[key] = nc

    iota_np = np.zeros((128, wmax2 + 256), dtype=np.float16)
    iota_np[:, :wmax2] = np.arange(wmax2, dtype=np.float16)
    ones_np = np.zeros((128, 6, 32), dtype=np.float16)
    r = np.arange(128)
    for v in range(4):
        ones_np[r, v, 8 * v + r // C] = 1.0
    for u in range(2):
        ones_np[r, 4 + u, 16 * u + r // C2] = 1.0
    ident_np = np.eye(128, dtype=np.float16)
    wt_np = np.ascontiguousarray(W.T).astype(np.float16)
    bb_np = np.ascontiguousarray(np.tile(b, (128, 1)), dtype=np.float32)

    in_maps = []
    for i in range(N_CORES):
        n_ov = len(ovsegs[i])
        xov = np.zeros((128, ovk, H), dtype=np.float16)
        ro = np.arange(n_ov)
        xov[ro % 128, ro // 128] = ovs[i]
        ovidx = np.full((128, ovk), PAD_IDX, dtype=np.float32)
        if ov_chunks:
            ovidx[ro % 128, ro // 128] = ovsegs[i] - np.repeat(wins, 128)[:n_ov]
        invc_np = np.ascontiguousarray(
            inv[S_PER * i : S_PER * (i + 1)].reshape(4, 128).T
        )
        in_maps.append(
            {
                "xb": bands[i],
                "xb2": bands2[i],
                "xov": xov,
                "ovidx": ovidx,
                "iota": iota_np,
                "ones32": ones_np,
                "ident": ident_np,
                "wt": wt_np,
                "invc": invc_np,
                "bb": bb_np,
            }
        )

    res = run_bass_kernel_spmd(nc, in_maps, core_ids=list(range(N_CORES)))
    return np.concatenate([res.results[i]["out"] for i in range(N_CORES)], axis=0)



# revision 3
# speedup vs baseline: 1.2543x; 1.2543x over previous
"""Segment-mean pooling (segment_sum / counts) + Linear, on 8 TRN2 NeuronCores.

Strategy: segment-ownership sharding with rank-slice packing.

The host sorts segments by count and deals them round-robin across the 8
cores (so per-core load is balanced).  Within a core its 512 segments are
kept count-sorted and split into 4 PSUM blocks of 128.  The host packs
the rows of x so that chunk k of block b holds, in SBUF partition p, the
k-th row of block b's p-th segment (zero if that segment has fewer than
k rows).  Segment-summing a chunk is then ONE full-width TensorE matmul
with a constant fp8 identity as the stationary operand: psum[p, :] +=
chunk[p, :].  Full 128-wide matmuls keep the PE clock warm (HAM) and
need no per-row index handling, no one-hots, and no overflow pass.

x is shipped as fp8 e4m3 with per-segment error-feedback quantization:
rows of one segment are quantized in sequence, carrying the running
quantization residual into the next row, so the device-side segment sum
telescopes to full-precision accuracy minus ONE element's rounding
(final rel err ~5e-3 vs the 2e-2 gate) at half the f16 DMA bytes.

Two consecutive chunks share one matmul (moving operand [128, 2, 256] =
512 fp8 columns into a full [128, 512] f32 PSUM bank); the two column
halves are folded by a single DVE add per block in the epilogue.

Epilogue per block: fold halves -> pooled f16, PE-transpose to [h, s],
Linear via 2 accumulated matmuls against W.T, then scale rows by
1/(count+eps) with fused bias-add, DMA out as f16 (host upcasts and
unpermutes segments).
"""

import numpy as np
import ml_dtypes

import concourse.bass as bass
import concourse.mybir as mybir
from concourse.bass_utils import run_bass_kernel_spmd

N_CORES = 8
S_TOTAL = 4096
S_PER = S_TOTAL // N_CORES  # 512 segments per core
NBLK = 4  # PSUM blocks of 128 segments per core
H = 256
EPS = np.float32(1e-8)
F8 = ml_dtypes.float8_e4m3  # matches mybir.dt.float8e4

N_WARM = 24  # PE warmup matmuls (HAM clock ramp)

_graph_cache: dict = {}


def _build(nb: tuple) -> "bass.Bass":
    """nb[b] = chunk count of block b (even, same on every core)."""
    f8 = mybir.dt.float8e4
    f16 = mybir.dt.float16
    f32 = mybir.dt.float32
    ktot = sum(nb)
    off = [sum(nb[:b]) for b in range(NBLK)]

    nc = bass.Bass()

    xp_d = nc.declare_dram_parameter("xp", [128, ktot, H], f8, isOutput=False)
    id8_d = nc.declare_dram_parameter("id8", [128, 128], f8, isOutput=False)
    idh_d = nc.declare_dram_parameter("idh", [128, 128], f16, isOutput=False)
    wt_d = nc.declare_dram_parameter("wt", [H, H], f16, isOutput=False)
    invc_d = nc.declare_dram_parameter("invc", [128, NBLK], f32, isOutput=False)
    bb_d = nc.declare_dram_parameter("bb", [128, H], f32, isOutput=False)
    out_d = nc.declare_dram_parameter("out", [S_PER, H], f16, isOutput=True)

    from contextlib import ExitStack

    with ExitStack() as ctx:
        xbb = ctx.enter_context(nc.sbuf_tensor("xbb", [128, ktot, H], f8))
        id8_sb = ctx.enter_context(nc.sbuf_tensor("id8_sb", [128, 128], f8))
        idh_sb = ctx.enter_context(nc.sbuf_tensor("idh_sb", [128, 128], f16))
        wt_sb = ctx.enter_context(nc.sbuf_tensor("wt_sb", [128, 2, H], f16))
        invc_sb = ctx.enter_context(nc.sbuf_tensor("invc_sb", [128, NBLK], f32))
        bb_sb = ctx.enter_context(nc.sbuf_tensor("bb_sb", [128, H], f32))
        pool_sb = ctx.enter_context(nc.sbuf_tensor("pool_sb", [128, NBLK, H], f16))
        sums2_sb = ctx.enter_context(
            nc.sbuf_tensor("sums2_sb", [128, 2, S_PER], f16)
        )
        out_sb = ctx.enter_context(nc.sbuf_tensor("out_sb", [128, NBLK, H], f16))
        # every PSUM tensor padded to one full private 2 KiB bank
        ps = [
            ctx.enter_context(nc.psum_tensor(f"ps{b}", [128, 512], f32))
            for b in range(NBLK)
        ]
        ps_t = [
            ctx.enter_context(nc.psum_tensor(f"ps_t{i}", [128, 1024], f16))
            for i in range(2)
        ]
        ps_x = ctx.enter_context(nc.psum_tensor("ps_x", [128, 512], f32))
        dma_sem = ctx.enter_context(nc.semaphore("dma_sem"))
        csem = {
            name: ctx.enter_context(nc.semaphore(f"csem_{name}"))
            for name in ("id8", "idh", "wt", "invc", "bb")
        }
        xsem = [
            ctx.enter_context(nc.semaphore(f"xsem{b}")) for b in range(NBLK)
        ]
        mmf_sem = ctx.enter_context(nc.semaphore("mmf_sem"))
        fold_sem = ctx.enter_context(nc.semaphore("fold_sem"))
        tr_sem = ctx.enter_context(nc.semaphore("tr_sem"))
        cp2_sem = ctx.enter_context(nc.semaphore("cp2_sem"))
        lin_sem = ctx.enter_context(nc.semaphore("lin_sem"))
        oe_sem = ctx.enter_context(nc.semaphore("oe_sem"))
        block = ctx.enter_context(nc.Block())

        @block.sync
        def _(sync):
            # consts in consumption order; idh first (PE warmup needs it)
            sync.dma_start(out=idh_sb[:, :], in_=idh_d[:, :]).then_inc(
                csem["idh"], 16
            )
            sync.dma_start(out=id8_sb[:, :], in_=id8_d[:, :]).then_inc(
                csem["id8"], 16
            )
            sync.dma_start(
                out=wt_sb[:, :, :],
                in_=wt_d[:, :].rearrange("(t p) j -> p t j", p=128),
            ).then_inc(csem["wt"], 16)
            sync.dma_start(out=invc_sb[:, :], in_=invc_d[:, :]).then_inc(
                csem["invc"], 16
            )
            sync.dma_start(out=bb_sb[:, :], in_=bb_d[:, :]).then_inc(
                csem["bb"], 16
            )
            for b in range(NBLK):
                sync.wait_ge(oe_sem, b + 1)
                sync.dma_start(
                    out=out_d[128 * b : 128 * (b + 1), :], in_=out_sb[:, b, :]
                ).then_inc(dma_sem, 16)
            for name in ("id8", "idh", "wt", "invc", "bb"):
                sync.wait_ge(csem[name], 16)
            sync.wait_ge(dma_sem, 16 * NBLK)

        @block.scalar
        def _(scalar):
            for b in range(NBLK):
                scalar.dma_start(
                    out=xbb[:, off[b] : off[b] + nb[b], :],
                    in_=xp_d[:, off[b] : off[b] + nb[b], :],
                ).then_inc(xsem[b], 16)
            for b in range(NBLK):
                scalar.wait_ge(xsem[b], 16)

        @block.vector
        def _(vector):
            for b in range(NBLK):
                vector.wait_ge(mmf_sem, b + 1)
                # fold the two column halves: pooled_b = ps[:,0:256]+ps[:,256:512]
                # (DVE can read only ONE input from PSUM per instruction)
                vector.tensor_copy(
                    out=pool_sb[:, b, :], in_=ps[b][:, H : 2 * H]
                )
                vector.scalar_tensor_tensor(
                    out=pool_sb[:, b, :],
                    in0=ps[b][:, 0:H],
                    scalar=1.0,
                    in1=pool_sb[:, b, :],
                    op0=mybir.AluOpType.mult,
                    op1=mybir.AluOpType.add,
                ).then_inc(fold_sem, 1)
            for b in range(NBLK):
                vector.wait_ge(tr_sem, b + 1)
                vector.tensor_copy(
                    out=sums2_sb[:, 0, 128 * b : 128 * (b + 1)],
                    in_=ps_t[b % 2][:, 0:128],
                )
                vector.tensor_copy(
                    out=sums2_sb[:, 1, 128 * b : 128 * (b + 1)],
                    in_=ps_t[b % 2][:, 128:256],
                ).then_inc(cp2_sem, 1)
            vector.wait_ge(csem["invc"], 16)
            vector.wait_ge(csem["bb"], 16)
            for b in range(NBLK):
                vector.wait_ge(lin_sem, b + 1)
                vector.scalar_tensor_tensor(
                    out=out_sb[:, b, :],
                    in0=ps[b][:, 0:H],
                    scalar=invc_sb[:, b : b + 1],
                    in1=bb_sb[:, :],
                    op0=mybir.AluOpType.mult,
                    op1=mybir.AluOpType.add,
                ).then_inc(oe_sem, 1)

        @block.tensor
        def _(tensor):
            # HAM warmup: sustained full-width activity from the earliest
            # moment so the PE is at 2.4 GHz when the first data lands
            tensor.wait_ge(csem["idh"], 16)
            for _ in range(N_WARM):
                tensor.matmul(
                    ps_x[:, 0:128], idh_sb[:, :], idh_sb[:, :],
                    start=True, stop=True, skip_group_check=True,
                )
            tensor.wait_ge(csem["id8"], 16)
            # rank-slice accumulation: psum[p, :] += chunk[p, :]
            for b in range(NBLK):
                tensor.wait_ge(xsem[b], 16)
                npair = nb[b] // 2
                for j in range(npair):
                    tensor.matmul(
                        ps[b][:, 0:512],
                        id8_sb[:, :],
                        xbb[:, off[b] + 2 * j : off[b] + 2 * j + 2, :],
                        start=(j == 0),
                        stop=(j == npair - 1),
                        skip_group_check=True,
                    )
                # fence: a matmul's then_inc can fire before its PSUM
                # writes drain; hand the bank to DVE only after a
                # trailing fence matmul completes
                tensor.matmul(
                    ps_x[:, 0:128], idh_sb[:, :], idh_sb[:, :],
                    start=True, stop=True, skip_group_check=True,
                ).then_inc(mmf_sem, 1)
            # transposes: pooled [s, h] -> pooled_T [h, s], per block
            for b in range(NBLK):
                tensor.wait_ge(fold_sem, b + 1)
                if b >= 2:
                    # ps_t[b%2] is read by DVE for block b-2; don't
                    # overwrite until that copy is done
                    tensor.wait_ge(cp2_sem, b - 1)
                for hb in range(2):
                    tensor.transpose(
                        ps_t[b % 2][:, 128 * hb : 128 * (hb + 1)],
                        pool_sb[:, b, 128 * hb : 128 * (hb + 1)],
                        idh_sb[:, :],
                    )
                tensor.matmul(
                    ps_x[:, 0:128], idh_sb[:, :], idh_sb[:, :],
                    start=True, stop=True, skip_group_check=True,
                ).then_inc(tr_sem, 1)
            # Linear: out[s, j] = sum_h pooled_T[h, s] * wt[h, j]
            tensor.wait_ge(csem["wt"], 16)
            for b in range(NBLK):
                tensor.wait_ge(cp2_sem, b + 1)
                tensor.matmul(
                    ps[b][:, 0:H],
                    sums2_sb[:, 0, 128 * b : 128 * (b + 1)],
                    wt_sb[:, 0, :],
                    start=True,
                    stop=False,
                    skip_group_check=True,
                )
                tensor.matmul(
                    ps[b][:, 0:H],
                    sums2_sb[:, 1, 128 * b : 128 * (b + 1)],
                    wt_sb[:, 1, :],
                    start=False,
                    stop=True,
                    skip_group_check=True,
                )
                tensor.matmul(
                    ps_x[:, 0:128], idh_sb[:, :], idh_sb[:, :],
                    start=True, stop=True, skip_group_check=True,
                ).then_inc(lin_sem, 1)

    return nc


def kernel(x, dst_idx, dst_size, W, b):
    x = np.asarray(x, dtype=np.float32)
    idx = np.asarray(dst_idx).astype(np.int64)
    W = np.asarray(W, dtype=np.float32)
    b = np.asarray(b, dtype=np.float32)
    S = int(dst_size)
    assert S == S_TOTAL and x.shape[1] == H

    counts = np.bincount(idx, minlength=S)
    inv = (np.float32(1.0) / (counts + EPS)).astype(np.float32)

    # deal count-sorted segments round-robin across cores; within a core
    # they stay count-sorted (ascending) -> blocks of 128 have near-equal
    # counts, so rank-slice padding is small
    seg_order = np.argsort(counts, kind="stable")  # [4096] ascending count
    # seg_core[s], seg_pos[s]: placement of segment s
    seg_core = np.empty(S, dtype=np.int64)
    seg_pos = np.empty(S, dtype=np.int64)
    seg_core[seg_order] = np.arange(S) % N_CORES
    seg_pos[seg_order] = np.arange(S) // N_CORES

    # per-(core, block) chunk counts -> shared schedule = max over cores,
    # rounded up to even (chunks are consumed in pairs)
    core_segs = [seg_order[c::N_CORES] for c in range(N_CORES)]  # sorted asc
    nb = []
    for blk in range(NBLK):
        m = max(int(counts[core_segs[c][128 * blk : 128 * (blk + 1)]].max())
                for c in range(N_CORES))
        nb.append(m + (m % 2))
    nb = tuple(nb)
    ktot = sum(nb)
    off = [sum(nb[:blk]) for blk in range(NBLK)]

    nc = _graph_cache.get(nb)
    if nc is None:
        nc = _build(nb)
        _graph_cache[nb] = nc

    # error-feedback fp8 quantization in segment-rank order: the running
    # residual of each (segment, feature) is carried into the next row,
    # so the segment sum telescopes to ~one element's rounding error
    order = np.argsort(idx, kind="stable")
    sidx = idx[order]
    starts = np.searchsorted(sidx, np.arange(S + 1))
    rank = np.arange(len(sidx)) - starts[sidx]
    xq = np.empty((len(idx), H), dtype=F8)
    err = np.zeros((S, H), dtype=np.float32)
    maxrank = int(rank.max())
    for r in range(maxrank + 1):
        sel = rank == r
        rows = order[sel]
        segs = sidx[sel]
        v = x[rows] + err[segs]
        q = v.astype(F8)
        err[segs] = v - q.astype(np.float32)
        xq[rows] = q

    # pack rank-slice chunks: xp[pos, off[blk] + rank, :] = row
    row_core = seg_core[sidx]
    row_pos = seg_pos[sidx]
    row_blk = row_pos // 128
    row_p = row_pos % 128
    row_chunk = np.asarray(off, dtype=np.int64)[row_blk] + rank

    id8_np = np.eye(128, dtype=F8)
    idh_np = np.eye(128, dtype=np.float16)
    wt_np = np.ascontiguousarray(W.T).astype(np.float16)
    bb_np = np.ascontiguousarray(np.tile(b, (128, 1)), dtype=np.float32)

    in_maps = []
    for c in range(N_CORES):
        m = row_core == c
        xp = np.zeros((128, ktot, H), dtype=F8)
        xp[row_p[m], row_chunk[m]] = xq[order[m]]
        invc_np = np.ascontiguousarray(
            inv[core_segs[c]].reshape(NBLK, 128).T
        )
        in_maps.append(
            {
                "xp": xp,
                "id8": id8_np,
                "idh": idh_np,
                "wt": wt_np,
                "invc": invc_np,
                "bb": bb_np,
            }
        )

    res = run_bass_kernel_spmd(nc, in_maps, core_ids=list(range(N_CORES)))
    out = np.empty((S, H), dtype=np.float32)
    for c in range(N_CORES):
        out[core_segs[c]] = res.results[c]["out"].astype(np.float32)
    return out


# revision 10
# speedup vs baseline: 1.3585x; 1.0831x over previous
"""Segment-mean pooling (segment_sum / counts) + Linear, on 8 TRN2 NeuronCores.

Strategy: segment-ownership sharding with rank-slice packing and fp8
DoubleRow matmuls.

The host sorts segments by count and deals them round-robin across the 8
cores (so per-core load is balanced).  Within a core its 512 segments are
kept count-sorted and split into 4 PSUM blocks of 128.  The host packs
the rows of x so that chunk c of block b holds, byte-interleaved in SBUF
partition p, rows 2c and 2c+1 of block b's p-th segment (zeros where the
segment has fewer rows).  Segment-summing a chunk is then ONE fp8
DoubleRow TensorE matmul (256 rows per ~109 ns) with a constant doubled
identity as the stationary operand: psum[p, :] += row2c[p, :] +
row2c1[p, :].  Full 128-wide matmuls keep the PE clock warm (HAM) and
need no per-row index handling, no one-hots, and no overflow pass.

x is shipped as fp8 e4m3 with per-segment error-feedback quantization:
rows of one segment are quantized in sequence, carrying the running
quantization residual into the next row, so the device-side segment sum
telescopes to full-precision accuracy minus ONE element's rounding
(final rel err ~5e-3 vs the 2e-2 gate) at half the f16 DMA bytes.  The
kernel is DMA-bound at ~350 GB/s; x streams in ~0.4 MB groups whose
completions are tracked with one cumulative semaphore (per-queue FIFO).

Epilogue per block: cast sums to f16, PE-transpose to [h, s], Linear via
2 accumulated matmuls against W.T, scale rows by 1/(count+eps) with
fused bias-add, one f16 output DMA (host upcasts and unpermutes).
"""

import numpy as np
import ml_dtypes

import concourse.bass as bass
import concourse.mybir as mybir
from concourse.bass_utils import run_bass_kernel_spmd

N_CORES = 8
S_TOTAL = 4096
S_PER = S_TOTAL // N_CORES  # 512 segments per core
NBLK = 4  # PSUM blocks of 128 segments per core
H = 256
EPS = np.float32(1e-8)
F8 = ml_dtypes.float8_e4m3  # matches mybir.dt.float8e4

N_WARM = 30  # PE warmup matmuls (HAM clock ramp)
GCH = 6  # x DMA group size in DoubleRow chunks (6 * 64 KB = 384 KB)

_graph_cache: dict = {}


def _groups(nb2):
    """Split per-block chunk counts into DMA groups of <= GCH chunks.

    Returns a list of (start_chunk, n_chunks) in global chunk indexing.
    """
    gs = []
    base = 0
    for n in nb2:
        c = 0
        while c < n:
            g = min(GCH, n - c)
            gs.append((base + c, g))
            c += g
        base += n
    return gs


def _build(nb2: tuple) -> "bass.Bass":
    """nb2[b] = DoubleRow chunk count of block b (same on every core)."""
    f8 = mybir.dt.float8e4
    f16 = mybir.dt.float16
    f32 = mybir.dt.float32
    ktot = sum(nb2)
    off = [sum(nb2[:b]) for b in range(NBLK)]
    groups = _groups(nb2)
    # group index that completes chunk c, for wait thresholds
    g_of_chunk = {}
    for gi, (c0, n) in enumerate(groups):
        for c in range(c0, c0 + n):
            g_of_chunk[c] = gi

    nc = bass.Bass()

    xp_d = nc.declare_dram_parameter("xp", [128, ktot, 2 * H], f8, isOutput=False)
    did_d = nc.declare_dram_parameter("did", [128, 2, 128], f8, isOutput=False)
    idh_d = nc.declare_dram_parameter("idh", [128, 128], f16, isOutput=False)
    wt_d = nc.declare_dram_parameter("wt", [H, H], f16, isOutput=False)
    invc_d = nc.declare_dram_parameter("invc", [128, NBLK], f32, isOutput=False)
    bb_d = nc.declare_dram_parameter("bb", [128, H], f32, isOutput=False)
    out_d = nc.declare_dram_parameter("out", [S_PER, H], f16, isOutput=True)

    from contextlib import ExitStack

    with ExitStack() as ctx:
        xbb = ctx.enter_context(nc.sbuf_tensor("xbb", [128, ktot, 2 * H], f8))
        did_sb = ctx.enter_context(nc.sbuf_tensor("did_sb", [128, 2, 128], f8))
        idh_sb = ctx.enter_context(nc.sbuf_tensor("idh_sb", [128, 128], f16))
        wt_sb = ctx.enter_context(nc.sbuf_tensor("wt_sb", [128, 2, H], f16))
        invc_sb = ctx.enter_context(nc.sbuf_tensor("invc_sb", [128, NBLK], f32))
        bb_sb = ctx.enter_context(nc.sbuf_tensor("bb_sb", [128, H], f32))
        pool_sb = ctx.enter_context(nc.sbuf_tensor("pool_sb", [128, NBLK, H], f16))
        sums2_sb = ctx.enter_context(
            nc.sbuf_tensor("sums2_sb", [128, 2, S_PER], f16)
        )
        out_sb = ctx.enter_context(nc.sbuf_tensor("out_sb", [128, NBLK, H], f16))
        # every PSUM tensor padded to one full private 2 KiB bank
        ps = [
            ctx.enter_context(nc.psum_tensor(f"ps{b}", [128, 512], f32))
            for b in range(NBLK)
        ]
        ps_t = [
            ctx.enter_context(nc.psum_tensor(f"ps_t{i}", [128, 1024], f16))
            for i in range(2)
        ]
        ps_x = ctx.enter_context(nc.psum_tensor("ps_x", [128, 512], f32))
        dma_sem = ctx.enter_context(nc.semaphore("dma_sem"))
        csem = {
            name: ctx.enter_context(nc.semaphore(f"csem_{name}"))
            for name in ("did", "idh", "wt", "invc", "bb")
        }
        xsem = [
            ctx.enter_context(nc.semaphore(f"xsem{g}"))
            for g in range(len(groups))
        ]
        mmf_sem = ctx.enter_context(nc.semaphore("mmf_sem"))
        fold_sem = ctx.enter_context(nc.semaphore("fold_sem"))
        tr_sem = ctx.enter_context(nc.semaphore("tr_sem"))
        cp2_sem = ctx.enter_context(nc.semaphore("cp2_sem"))
        lin_sem = ctx.enter_context(nc.semaphore("lin_sem"))
        oe_sem = ctx.enter_context(nc.semaphore("oe_sem"))
        block = ctx.enter_context(nc.Block())

        @block.sync
        def _(sync):
            # consts in consumption order; idh first (PE warmup needs it)
            sync.dma_start(out=idh_sb[:, :], in_=idh_d[:, :]).then_inc(
                csem["idh"], 16
            )
            sync.dma_start(out=did_sb[:, :, :], in_=did_d[:, :, :]).then_inc(
                csem["did"], 16
            )
            sync.dma_start(
                out=wt_sb[:, :, :],
                in_=wt_d[:, :].rearrange("(t p) j -> p t j", p=128),
            ).then_inc(csem["wt"], 16)
            sync.dma_start(out=invc_sb[:, :], in_=invc_d[:, :]).then_inc(
                csem["invc"], 16
            )
            sync.dma_start(out=bb_sb[:, :], in_=bb_d[:, :]).then_inc(
                csem["bb"], 16
            )
            sync.wait_ge(oe_sem, NBLK)
            sync.dma_start(
                out=out_d[:, :].rearrange("(b p) j -> p b j", p=128),
                in_=out_sb[:, :, :],
            ).then_inc(dma_sem, 16)
            for name in ("did", "idh", "wt", "invc", "bb"):
                sync.wait_ge(csem[name], 16)
            sync.wait_ge(dma_sem, 16)

        @block.scalar
        def _(scalar):
            # one semaphore per group: groups land out of order (DMAs
            # round-robin across queues), so a shared cumulative
            # threshold can't tell WHICH group completed
            for gi, (c0, n) in enumerate(groups):
                scalar.dma_start(
                    out=xbb[:, c0 : c0 + n, :],
                    in_=xp_d[:, c0 : c0 + n, :],
                ).then_inc(xsem[gi], 16)
            for gi in range(len(groups)):
                scalar.wait_ge(xsem[gi], 16)

        @block.vector
        def _(vector):
            for b in range(NBLK):
                vector.wait_ge(mmf_sem, b + 1)
                vector.tensor_copy(
                    out=pool_sb[:, b, :], in_=ps[b][:, 0:H]
                ).then_inc(fold_sem, 1)
            for b in range(NBLK):
                vector.wait_ge(tr_sem, b + 1)
                vector.tensor_copy(
                    out=sums2_sb[:, 0, 128 * b : 128 * (b + 1)],
                    in_=ps_t[b % 2][:, 0:128],
                )
                vector.tensor_copy(
                    out=sums2_sb[:, 1, 128 * b : 128 * (b + 1)],
                    in_=ps_t[b % 2][:, 128:256],
                ).then_inc(cp2_sem, 1)
            vector.wait_ge(csem["invc"], 16)
            vector.wait_ge(csem["bb"], 16)
            for b in range(NBLK):
                vector.wait_ge(lin_sem, b + 1)
                vector.scalar_tensor_tensor(
                    out=out_sb[:, b, :],
                    in0=ps[b][:, 0:H],
                    scalar=invc_sb[:, b : b + 1],
                    in1=bb_sb[:, :],
                    op0=mybir.AluOpType.mult,
                    op1=mybir.AluOpType.add,
                ).then_inc(oe_sem, 1)

        @block.tensor
        def _(tensor):
            # HAM warmup: sustained activity from the earliest moment so
            # the PE is at 2.4 GHz when the first data group lands
            tensor.wait_ge(csem["idh"], 16)
            for _ in range(N_WARM):
                tensor.matmul(
                    ps_x[:, 0:128], idh_sb[:, :], idh_sb[:, :],
                    start=True, stop=True, skip_group_check=True,
                )
            tensor.wait_ge(csem["did"], 16)
            # rank-slice accumulation, 2 rank slices per DoubleRow matmul:
            # psum[p, :] += chunk[p, even, :] + chunk[p, odd, :]
            waited = -1
            for b in range(NBLK):
                for j in range(nb2[b]):
                    c = off[b] + j
                    gi = g_of_chunk[c]
                    if gi > waited:
                        tensor.wait_ge(xsem[gi], 16)
                        waited = gi
                    tensor.matmul(
                        ps[b][:, 0:H],
                        did_sb[:, :, :],
                        xbb[:, c, :].rearrange("p (n o) -> p o n", o=2),
                        start=(j == 0),
                        stop=(j == nb2[b] - 1),
                        skip_group_check=True,
                        perf_mode=mybir.MatmulPerfMode.DoubleRow,
                    )
                # fence: a matmul's then_inc can fire before its PSUM
                # writes drain; hand the bank to DVE only after a
                # trailing fence matmul completes
                tensor.matmul(
                    ps_x[:, 0:128], idh_sb[:, :], idh_sb[:, :],
                    start=True, stop=True, skip_group_check=True,
                ).then_inc(mmf_sem, 1)
            # transposes: pooled [s, h] -> pooled_T [h, s], per block
            for b in range(NBLK):
                tensor.wait_ge(fold_sem, b + 1)
                if b >= 2:
                    # ps_t[b%2] is read by DVE for block b-2; don't
                    # overwrite until that copy is done
                    tensor.wait_ge(cp2_sem, b - 1)
                for hb in range(2):
                    tensor.transpose(
                        ps_t[b % 2][:, 128 * hb : 128 * (hb + 1)],
                        pool_sb[:, b, 128 * hb : 128 * (hb + 1)],
                        idh_sb[:, :],
                    )
                tensor.matmul(
                    ps_x[:, 0:128], idh_sb[:, :], idh_sb[:, :],
                    start=True, stop=True, skip_group_check=True,
                ).then_inc(tr_sem, 1)
            # Linear: out[s, j] = sum_h pooled_T[h, s] * wt[h, j]
            tensor.wait_ge(csem["wt"], 16)
            for b in range(NBLK):
                tensor.wait_ge(cp2_sem, b + 1)
                tensor.matmul(
                    ps[b][:, 0:H],
                    sums2_sb[:, 0, 128 * b : 128 * (b + 1)],
                    wt_sb[:, 0, :],
                    start=True,
                    stop=False,
                    skip_group_check=True,
                )
                tensor.matmul(
                    ps[b][:, 0:H],
                    sums2_sb[:, 1, 128 * b : 128 * (b + 1)],
                    wt_sb[:, 1, :],
                    start=False,
                    stop=True,
                    skip_group_check=True,
                )
                tensor.matmul(
                    ps_x[:, 0:128], idh_sb[:, :], idh_sb[:, :],
                    start=True, stop=True, skip_group_check=True,
                ).then_inc(lin_sem, 1)

    return nc


def kernel(x, dst_idx, dst_size, W, b):
    x = np.asarray(x, dtype=np.float32)
    idx = np.asarray(dst_idx).astype(np.int64)
    W = np.asarray(W, dtype=np.float32)
    b = np.asarray(b, dtype=np.float32)
    S = int(dst_size)
    assert S == S_TOTAL and x.shape[1] == H

    counts = np.bincount(idx, minlength=S)
    inv = (np.float32(1.0) / (counts + EPS)).astype(np.float32)

    # deal count-sorted segments round-robin across cores; within a core
    # they stay count-sorted (ascending) -> blocks of 128 have near-equal
    # counts, so rank-slice padding is small
    seg_order = np.argsort(counts, kind="stable")  # [4096] ascending count
    seg_core = np.empty(S, dtype=np.int64)
    seg_pos = np.empty(S, dtype=np.int64)
    seg_core[seg_order] = np.arange(S) % N_CORES
    seg_pos[seg_order] = np.arange(S) // N_CORES

    # per-(core, block) max count -> shared schedule = max over cores, in
    # DoubleRow chunks of 2 rank slices each
    core_segs = [seg_order[c::N_CORES] for c in range(N_CORES)]  # sorted asc
    nb2 = []
    for blk in range(NBLK):
        m = max(int(counts[core_segs[c][128 * blk : 128 * (blk + 1)]].max())
                for c in range(N_CORES))
        nb2.append((m + 1) // 2)
    nb2 = tuple(nb2)
    ktot = sum(nb2)
    off = [sum(nb2[:blk]) for blk in range(NBLK)]

    nc = _graph_cache.get(nb2)
    if nc is None:
        nc = _build(nb2)
        _graph_cache[nb2] = nc

    # error-feedback fp8 quantization in segment-rank order: the running
    # residual of each (segment, feature) is carried into the next row,
    # so the segment sum telescopes to ~one element's rounding error
    order = np.argsort(idx, kind="stable")
    sidx = idx[order]
    starts = np.searchsorted(sidx, np.arange(S + 1))
    rank = np.arange(len(sidx)) - starts[sidx]
    xq = np.empty((len(idx), H), dtype=F8)
    err = np.zeros((S, H), dtype=np.float32)
    maxrank = int(rank.max())
    for r in range(maxrank + 1):
        sel = rank == r
        rows = order[sel]
        segs = sidx[sel]
        v = x[rows] + err[segs]
        q = v.astype(F8)
        err[segs] = v - q.astype(np.float32)
        xq[rows] = q

    # pack DoubleRow chunks: view [128, ktot, 256 features, 2 rank-parity]
    row_core = seg_core[sidx]
    row_pos = seg_pos[sidx]
    row_blk = row_pos // 128
    row_p = row_pos % 128
    row_chunk = np.asarray(off, dtype=np.int64)[row_blk] + rank // 2
    row_o = rank % 2

    did_np = np.zeros((128, 2, 128), dtype=F8)
    r128 = np.arange(128)
    did_np[r128, 0, r128] = 1.0
    did_np[r128, 1, r128] = 1.0
    idh_np = np.eye(128, dtype=np.float16)
    wt_np = np.ascontiguousarray(W.T).astype(np.float16)
    bb_np = np.ascontiguousarray(np.tile(b, (128, 1)), dtype=np.float32)

    in_maps = []
    for c in range(N_CORES):
        m = row_core == c
        xp = np.zeros((128, ktot, H, 2), dtype=F8)
        xp[row_p[m], row_chunk[m], :, row_o[m]] = xq[order[m]]
        invc_np = np.ascontiguousarray(
            inv[core_segs[c]].reshape(NBLK, 128).T
        )
        in_maps.append(
            {
                "xp": xp.reshape(128, ktot, 2 * H),
                "did": did_np,
                "idh": idh_np,
                "wt": wt_np,
                "invc": invc_np,
                "bb": bb_np,
            }
        )

    res = run_bass_kernel_spmd(nc, in_maps, core_ids=list(range(N_CORES)))
    out = np.empty((S, H), dtype=np.float32)
    for c in range(N_CORES):
        out[core_segs[c]] = res.results[c]["out"].astype(np.float32)
    return out


# revision 14
# speedup vs baseline: 1.3970x; 1.0283x over previous
"""Segment-mean pooling (segment_sum / counts) + Linear, on 8 TRN2 NeuronCores.

Strategy: segment-ownership sharding with rank-slice packing and fp8
DoubleRow matmuls.

The host sorts segments by count and deals them round-robin across the 8
cores (so per-core load is balanced).  Within a core its 512 segments are
kept count-sorted and split into 4 PSUM blocks of 128.  The host packs
the rows of x so that chunk c of block b holds, byte-interleaved in SBUF
partition p, rows 2c and 2c+1 of block b's p-th segment (zeros where the
segment has fewer rows).  Segment-summing a chunk is then ONE fp8
DoubleRow TensorE matmul (256 rows per ~109 ns) with a constant doubled
identity as the stationary operand: psum[p, :] += row2c[p, :] +
row2c1[p, :].  Full 128-wide matmuls keep the PE clock warm (HAM) and
need no per-row index handling, no one-hots, and no overflow pass.

x is shipped as fp8 e4m3 with per-segment error-feedback quantization:
rows of one segment are quantized in sequence, carrying the running
quantization residual into the next row, so the device-side segment sum
telescopes to full-precision accuracy minus ONE element's rounding
(final rel err ~5e-3 vs the 2e-2 gate) at half the f16 DMA bytes.  The
kernel is DMA-bound at ~350 GB/s; x streams in ~0.4 MB groups whose
completions are tracked with one cumulative semaphore (per-queue FIFO).

Epilogue per block: cast sums to f16, PE-transpose to [h, s], Linear via
2 accumulated matmuls against W.T, scale rows by 1/(count+eps) with
fused bias-add, one f16 output DMA (host upcasts and unpermutes).
"""

import numpy as np
import ml_dtypes

import concourse.bass as bass
import concourse.mybir as mybir
from concourse.bass_utils import run_bass_kernel_spmd

N_CORES = 8
S_TOTAL = 4096
S_PER = S_TOTAL // N_CORES  # 512 segments per core
NBLK = 4  # PSUM blocks of 128 segments per core
H = 256
EPS = np.float32(1e-8)
F8 = ml_dtypes.float8_e4m3  # matches mybir.dt.float8e4

N_WARM = 30  # PE warmup matmuls (HAM clock ramp)
GBIG = 12  # x DMA bulk group size in chunks (12 * 64 KB = 768 KB)

_graph_cache: dict = {}


def _groups(nb2):
    """Split the global chunk stream into DMA groups: big groups for
    bandwidth (transfers under ~0.5 MB lose HBM rate), two small tail
    groups so the PE trail after the last byte stays short.

    Returns a list of (start_chunk, n_chunks) in global chunk indexing.
    """
    total = sum(nb2)
    sizes = []
    left = total
    while left > GBIG:
        sizes.append(GBIG)
        left -= GBIG
    if left > 5:
        sizes.append(left - left // 2)
        sizes.append(left // 2)
    elif left:
        sizes.append(left)
    gs = []
    base = 0
    for n in sizes:
        gs.append((base, n))
        base += n
    return gs


def _build(nb2: tuple) -> "bass.Bass":
    """nb2[b] = DoubleRow chunk count of block b (same on every core)."""
    f8 = mybir.dt.float8e4
    f16 = mybir.dt.float16
    f32 = mybir.dt.float32
    ktot = sum(nb2)
    off = [sum(nb2[:b]) for b in range(NBLK)]
    groups = _groups(nb2)
    # group index that completes chunk c, for wait thresholds
    g_of_chunk = {}
    for gi, (c0, n) in enumerate(groups):
        for c in range(c0, c0 + n):
            g_of_chunk[c] = gi

    nc = bass.Bass()

    xp_d = nc.declare_dram_parameter("xp", [128, ktot, 2 * H], f8, isOutput=False)
    did_d = nc.declare_dram_parameter("did", [128, 2, 128], f8, isOutput=False)
    idh_d = nc.declare_dram_parameter("idh", [128, 128], f16, isOutput=False)
    wt_d = nc.declare_dram_parameter("wt", [H, H], f16, isOutput=False)
    invc_d = nc.declare_dram_parameter("invc", [128, NBLK], f32, isOutput=False)
    bb_d = nc.declare_dram_parameter("bb", [128, H], f32, isOutput=False)
    # [partition, block, H]: contiguous DMA; the host un-interleaves
    out_d = nc.declare_dram_parameter("out", [128, NBLK, H], f16, isOutput=True)

    from contextlib import ExitStack

    with ExitStack() as ctx:
        xbb = ctx.enter_context(nc.sbuf_tensor("xbb", [128, ktot, 2 * H], f8))
        did_sb = ctx.enter_context(nc.sbuf_tensor("did_sb", [128, 2, 128], f8))
        idh_sb = ctx.enter_context(nc.sbuf_tensor("idh_sb", [128, 128], f16))
        wt_sb = ctx.enter_context(nc.sbuf_tensor("wt_sb", [128, 2, H], f16))
        invc_sb = ctx.enter_context(nc.sbuf_tensor("invc_sb", [128, NBLK], f32))
        bb_sb = ctx.enter_context(nc.sbuf_tensor("bb_sb", [128, H], f32))
        pool_sb = ctx.enter_context(nc.sbuf_tensor("pool_sb", [128, NBLK, H], f16))
        sums2_sb = ctx.enter_context(
            nc.sbuf_tensor("sums2_sb", [128, 2, S_PER], f16)
        )
        out_sb = ctx.enter_context(nc.sbuf_tensor("out_sb", [128, NBLK, H], f16))
        # every PSUM tensor padded to one full private 2 KiB bank
        ps = [
            ctx.enter_context(nc.psum_tensor(f"ps{b}", [128, 512], f32))
            for b in range(NBLK)
        ]
        ps_t = [
            ctx.enter_context(nc.psum_tensor(f"ps_t{i}", [128, 1024], f16))
            for i in range(2)
        ]
        ps_x = ctx.enter_context(nc.psum_tensor("ps_x", [128, 512], f32))
        dma_sem = ctx.enter_context(nc.semaphore("dma_sem"))
        csem = {
            name: ctx.enter_context(nc.semaphore(f"csem_{name}"))
            for name in ("did", "idh", "wt", "invc", "bb")
        }
        xsem = [
            ctx.enter_context(nc.semaphore(f"xsem{g}"))
            for g in range(len(groups))
        ]
        mmf_sem = ctx.enter_context(nc.semaphore("mmf_sem"))
        fold_sem = ctx.enter_context(nc.semaphore("fold_sem"))
        tr_sem = ctx.enter_context(nc.semaphore("tr_sem"))
        cp2_sem = ctx.enter_context(nc.semaphore("cp2_sem"))
        lin_sem = ctx.enter_context(nc.semaphore("lin_sem"))
        oe_sem = ctx.enter_context(nc.semaphore("oe_sem"))
        block = ctx.enter_context(nc.Block())

        @block.sync
        def _(sync):
            # consts in consumption order; idh first (PE warmup needs it)
            sync.dma_start(out=idh_sb[:, :], in_=idh_d[:, :]).then_inc(
                csem["idh"], 16
            )
            sync.dma_start(out=did_sb[:, :, :], in_=did_d[:, :, :]).then_inc(
                csem["did"], 16
            )
            sync.dma_start(
                out=wt_sb[:, :, :],
                in_=wt_d[:, :].rearrange("(t p) j -> p t j", p=128),
            ).then_inc(csem["wt"], 16)
            sync.dma_start(out=invc_sb[:, :], in_=invc_d[:, :]).then_inc(
                csem["invc"], 16
            )
            sync.dma_start(out=bb_sb[:, :], in_=bb_d[:, :]).then_inc(
                csem["bb"], 16
            )
            sync.wait_ge(oe_sem, NBLK)
            sync.dma_start(
                out=out_d[:, :, :], in_=out_sb[:, :, :]
            ).then_inc(dma_sem, 16)
            for name in ("did", "idh", "wt", "invc", "bb"):
                sync.wait_ge(csem[name], 16)
            sync.wait_ge(dma_sem, 16)

        @block.scalar
        def _(scalar):
            # one semaphore per group: groups land out of order (DMAs
            # round-robin across queues), so a shared cumulative
            # threshold can't tell WHICH group completed
            for gi, (c0, n) in enumerate(groups):
                scalar.dma_start(
                    out=xbb[:, c0 : c0 + n, :],
                    in_=xp_d[:, c0 : c0 + n, :],
                ).then_inc(xsem[gi], 16)
            for gi in range(len(groups)):
                scalar.wait_ge(xsem[gi], 16)

        @block.vector
        def _(vector):
            for b in range(NBLK):
                vector.wait_ge(mmf_sem, b + 1)
                vector.tensor_copy(
                    out=pool_sb[:, b, :], in_=ps[b][:, 0:H]
                ).then_inc(fold_sem, 1)
            for b in range(NBLK):
                vector.wait_ge(tr_sem, b + 1)
                vector.tensor_copy(
                    out=sums2_sb[:, 0, 128 * b : 128 * (b + 1)],
                    in_=ps_t[b % 2][:, 0:128],
                )
                vector.tensor_copy(
                    out=sums2_sb[:, 1, 128 * b : 128 * (b + 1)],
                    in_=ps_t[b % 2][:, 128:256],
                ).then_inc(cp2_sem, 1)
            vector.wait_ge(csem["invc"], 16)
            vector.wait_ge(csem["bb"], 16)
            for b in range(NBLK):
                vector.wait_ge(lin_sem, b + 1)
                vector.scalar_tensor_tensor(
                    out=out_sb[:, b, :],
                    in0=ps[b][:, 0:H],
                    scalar=invc_sb[:, b : b + 1],
                    in1=bb_sb[:, :],
                    op0=mybir.AluOpType.mult,
                    op1=mybir.AluOpType.add,
                ).then_inc(oe_sem, 1)

        @block.tensor
        def _(tensor):
            # HAM warmup: sustained activity from the earliest moment so
            # the PE is at 2.4 GHz when the first data group lands
            tensor.wait_ge(csem["idh"], 16)
            for _ in range(N_WARM):
                tensor.matmul(
                    ps_x[:, 0:128], idh_sb[:, :], idh_sb[:, :],
                    start=True, stop=True, skip_group_check=True,
                )
            tensor.wait_ge(csem["did"], 16)
            # rank-slice accumulation, 2 rank slices per DoubleRow matmul:
            # psum[p, :] += chunk[p, even, :] + chunk[p, odd, :]
            waited = -1
            for b in range(NBLK):
                for j in range(nb2[b]):
                    c = off[b] + j
                    gi = g_of_chunk[c]
                    if gi > waited:
                        tensor.wait_ge(xsem[gi], 16)
                        waited = gi
                    tensor.matmul(
                        ps[b][:, 0:H],
                        did_sb[:, :, :],
                        xbb[:, c, :].rearrange("p (n o) -> p o n", o=2),
                        start=(j == 0),
                        stop=(j == nb2[b] - 1),
                        skip_group_check=True,
                        perf_mode=mybir.MatmulPerfMode.DoubleRow,
                    )
                # fence: a matmul's then_inc can fire before its PSUM
                # writes drain; hand the bank to DVE only after a
                # trailing fence matmul completes
                tensor.matmul(
                    ps_x[:, 0:128], idh_sb[:, :], idh_sb[:, :],
                    start=True, stop=True, skip_group_check=True,
                ).then_inc(mmf_sem, 1)
            # transposes: pooled [s, h] -> pooled_T [h, s], per block
            for b in range(NBLK):
                tensor.wait_ge(fold_sem, b + 1)
                if b >= 2:
                    # ps_t[b%2] is read by DVE for block b-2; don't
                    # overwrite until that copy is done
                    tensor.wait_ge(cp2_sem, b - 1)
                for hb in range(2):
                    tensor.transpose(
                        ps_t[b % 2][:, 128 * hb : 128 * (hb + 1)],
                        pool_sb[:, b, 128 * hb : 128 * (hb + 1)],
                        idh_sb[:, :],
                    )
                tensor.matmul(
                    ps_x[:, 0:128], idh_sb[:, :], idh_sb[:, :],
                    start=True, stop=True, skip_group_check=True,
                ).then_inc(tr_sem, 1)
            # Linear: out[s, j] = sum_h pooled_T[h, s] * wt[h, j]
            tensor.wait_ge(csem["wt"], 16)
            for b in range(NBLK):
                tensor.wait_ge(cp2_sem, b + 1)
                tensor.matmul(
                    ps[b][:, 0:H],
                    sums2_sb[:, 0, 128 * b : 128 * (b + 1)],
                    wt_sb[:, 0, :],
                    start=True,
                    stop=False,
                    skip_group_check=True,
                )
                tensor.matmul(
                    ps[b][:, 0:H],
                    sums2_sb[:, 1, 128 * b : 128 * (b + 1)],
                    wt_sb[:, 1, :],
                    start=False,
                    stop=True,
                    skip_group_check=True,
                )
                tensor.matmul(
                    ps_x[:, 0:128], idh_sb[:, :], idh_sb[:, :],
                    start=True, stop=True, skip_group_check=True,
                ).then_inc(lin_sem, 1)

    return nc


def kernel(x, dst_idx, dst_size, W, b):
    x = np.asarray(x, dtype=np.float32)
    idx = np.asarray(dst_idx).astype(np.int64)
    W = np.asarray(W, dtype=np.float32)
    b = np.asarray(b, dtype=np.float32)
    S = int(dst_size)
    assert S == S_TOTAL and x.shape[1] == H

    counts = np.bincount(idx, minlength=S)
    inv = (np.float32(1.0) / (counts + EPS)).astype(np.float32)

    # deal count-sorted segments round-robin across cores; within a core
    # they stay count-sorted (ascending) -> blocks of 128 have near-equal
    # counts, so rank-slice padding is small
    seg_order = np.argsort(counts, kind="stable")  # [4096] ascending count
    seg_core = np.empty(S, dtype=np.int64)
    seg_pos = np.empty(S, dtype=np.int64)
    seg_core[seg_order] = np.arange(S) % N_CORES
    seg_pos[seg_order] = np.arange(S) // N_CORES

    # per-(core, block) max count -> shared schedule = max over cores, in
    # DoubleRow chunks of 2 rank slices each
    core_segs = [seg_order[c::N_CORES] for c in range(N_CORES)]  # sorted asc
    nb2 = []
    for blk in range(NBLK):
        m = max(int(counts[core_segs[c][128 * blk : 128 * (blk + 1)]].max())
                for c in range(N_CORES))
        nb2.append((m + 1) // 2)
    nb2 = tuple(nb2)
    ktot = sum(nb2)
    off = [sum(nb2[:blk]) for blk in range(NBLK)]

    nc = _graph_cache.get(nb2)
    if nc is None:
        nc = _build(nb2)
        _graph_cache[nb2] = nc

    # error-feedback fp8 quantization in segment-rank order: the running
    # residual of each (segment, feature) is carried into the next row,
    # so the segment sum telescopes to ~one element's rounding error
    order = np.argsort(idx, kind="stable")
    sidx = idx[order]
    starts = np.searchsorted(sidx, np.arange(S + 1))
    rank = np.arange(len(sidx)) - starts[sidx]
    xq = np.empty((len(idx), H), dtype=F8)
    err = np.zeros((S, H), dtype=np.float32)
    maxrank = int(rank.max())
    for r in range(maxrank + 1):
        sel = rank == r
        rows = order[sel]
        segs = sidx[sel]
        v = x[rows] + err[segs]
        q = v.astype(F8)
        err[segs] = v - q.astype(np.float32)
        xq[rows] = q

    # pack DoubleRow chunks: view [128, ktot, 256 features, 2 rank-parity]
    row_core = seg_core[sidx]
    row_pos = seg_pos[sidx]
    row_blk = row_pos // 128
    row_p = row_pos % 128
    row_chunk = np.asarray(off, dtype=np.int64)[row_blk] + rank // 2
    row_o = rank % 2

    did_np = np.zeros((128, 2, 128), dtype=F8)
    r128 = np.arange(128)
    did_np[r128, 0, r128] = 1.0
    did_np[r128, 1, r128] = 1.0
    idh_np = np.eye(128, dtype=np.float16)
    wt_np = np.ascontiguousarray(W.T).astype(np.float16)
    bb_np = np.ascontiguousarray(np.tile(b, (128, 1)), dtype=np.float32)

    in_maps = []
    for c in range(N_CORES):
        m = row_core == c
        xp = np.zeros((128, ktot, H, 2), dtype=F8)
        xp[row_p[m], row_chunk[m], :, row_o[m]] = xq[order[m]]
        invc_np = np.ascontiguousarray(
            inv[core_segs[c]].reshape(NBLK, 128).T
        )
        in_maps.append(
            {
                "xp": xp.reshape(128, ktot, 2 * H),
                "did": did_np,
                "idh": idh_np,
                "wt": wt_np,
                "invc": invc_np,
                "bb": bb_np,
            }
        )

    res = run_bass_kernel_spmd(nc, in_maps, core_ids=list(range(N_CORES)))
    out = np.empty((S, H), dtype=np.float32)
    for c in range(N_CORES):
        # device wrote [partition, block, H]; segment = core_segs[c][128b+p]
        o = res.results[c]["out"].astype(np.float32)
        out[core_segs[c]] = o.transpose(1, 0, 2).reshape(S_PER, H)
    return out
